# revision 2
# baseline (speedup 1.0000x reference)
"""Causal self-attention (B=4, S=2048, D=1024, single head) on 8 TRN2 cores.

Sharding: core c handles batch b = c//2 with query-tile parity p = c%2 —
its 8 query tiles of 128 rows are the absolute 128-row tiles {2j+p}.
Interleaving parities balances causal work exactly: both cores of a pair
process kv extents of ceil((2j+2)/4) 512-wide groups per local tile j,
so the single SPMD program is uniform; per-core variation is data-only
(query rows and the additive causal mask for the last kv group).

All matmuls run in float32r (full-rate fp32 with ~14-bit mantissa):
  qT[o,s] = WqT.T @ xT        kT likewise       v[s,o] = xT.T @ WvT
  scores[sq,kv] = qT.T @ kT (+ identity.T @ mask on the last group)
  P = exp(scale*scores) with fused row-sum (ScalarE accum_out)
  PT = PE-transpose(P);  out[sq,o] = PT.T @ v;  out *= 1/rowsum
"""

import numpy as np

B, S, D = 4, 2048, 1024
DC = D // 128          # contraction chunks
NB = S // 128          # kv blocks per batch
NT = 8                 # q tiles per core
SCALE = 1.0 / np.sqrt(np.float32(D))
NEG = np.float32(-1e30)

_cache = {}


def _E(j):
    # kv extent for local tile j, in 512-wide groups (uniform across cores)
    return (j + 2) // 2


def _build():
    if "nc" in _cache:
        return _cache["nc"]

    import concourse.bacc as bacc
    import concourse.mybir as mybir
    import concourse.tile as tile

    f32 = mybir.dt.float32
    f32r = mybir.dt.float32r
    AF = mybir.ActivationFunctionType

    nc = bacc.Bacc("TRN2", target_bir_lowering=False, debug=False,
                   num_devices=8)
    xq_d = nc.dram_tensor("xq", [D, NT * 128], f32r, kind="ExternalInput").ap()
    xkv_d = nc.dram_tensor("xkv", [D, S], f32r, kind="ExternalInput").ap()
    wq_d = nc.dram_tensor("wq", [D, D], f32r, kind="ExternalInput").ap()
    wk_d = nc.dram_tensor("wk", [D, D], f32r, kind="ExternalInput").ap()
    wv_d = nc.dram_tensor("wv", [D, D], f32r, kind="ExternalInput").ap()
    masks_d = nc.dram_tensor("masks", [NT * 128, 512], f32r,
                             kind="ExternalInput").ap()
    ident_d = nc.dram_tensor("ident", [128, 128], f32r,
                             kind="ExternalInput").ap()
    out_d = nc.dram_tensor("out", [NT * 128, D], f32,
                           kind="ExternalOutput").ap()

    with tile.TileContext(nc) as tc:
        with (
            tc.tile_pool(name="persist", bufs=1) as persist,
            tc.tile_pool(name="dram", bufs=1, space="DRAM") as dramp,
        ):
            kT = persist.tile([128, DC * S], f32r)        # [o%128, oc*S+kv]
            qT = persist.tile([128, DC * NT * 128], f32r)  # [o%128, oc*1024+sq]
            ident = persist.tile([128, 128], f32r)
            nc.sync.dma_start(ident[:], ident_d)
            vtmp = dramp.tile([S, D], f32r)

            # ---- Phase A: q projection (qT stays resident) ----
            with (
                tc.tile_pool(name="wA", bufs=1) as wp,
                tc.tile_pool(name="xA", bufs=2) as xs,
                tc.tile_pool(name="psA", bufs=4, space="PSUM") as psp,
            ):
                wq = wp.tile([128, DC * D], f32r)  # [d%128, dc*D+o]
                nc.sync.dma_start(wq[:].rearrange("p (c o) -> p c o", c=DC),
                                  wq_d.rearrange("(c p) o -> p c o", p=128))
                for sg in range(2):
                    xq = xs.tile([128, DC * 512], f32r, tag="xq")
                    nc.sync.dma_start(
                        xq[:].rearrange("p (c s) -> p c s", c=DC),
                        xq_d[:, sg * 512:(sg + 1) * 512]
                        .rearrange("(c p) s -> p c s", p=128))
                    for ot in range(8):
                        ps = psp.tile([128, 512], f32, tag="pj")
                        for dc in range(DC):
                            nc.tensor.matmul(
                                ps[:],
                                wq[:, dc * D + ot * 128:dc * D + ot * 128 + 128],
                                xq[:, dc * 512:(dc + 1) * 512],
                                start=(dc == 0), stop=(dc == DC - 1))
                        nc.vector.tensor_copy(
                            qT[:, ot * 1024 + sg * 512:ot * 1024 + sg * 512 + 512],
                            ps[:])

            # ---- Phase B: k projection (kT stays resident) ----
            with (
                tc.tile_pool(name="wB", bufs=1) as wp,
                tc.tile_pool(name="xB", bufs=2) as xs,
                tc.tile_pool(name="psB", bufs=4, space="PSUM") as psp,
            ):
                wk = wp.tile([128, DC * D], f32r)
                nc.sync.dma_start(wk[:].rearrange("p (c o) -> p c o", c=DC),
                                  wk_d.rearrange("(c p) o -> p c o", p=128))
                for sg in range(4):
                    xkv = xs.tile([128, DC * 512], f32r, tag="xkv")
                    nc.sync.dma_start(
                        xkv[:].rearrange("p (c s) -> p c s", c=DC),
                        xkv_d[:, sg * 512:(sg + 1) * 512]
                        .rearrange("(c p) s -> p c s", p=128))
                    for ot in range(8):
                        ps = psp.tile([128, 512], f32, tag="pj")
                        for dc in range(DC):
                            nc.tensor.matmul(
                                ps[:],
                                wk[:, dc * D + ot * 128:dc * D + ot * 128 + 128],
                                xkv[:, dc * 512:(dc + 1) * 512],
                                start=(dc == 0), stop=(dc == DC - 1))
                        nc.vector.tensor_copy(
                            kT[:, ot * S + sg * 512:ot * S + sg * 512 + 512],
                            ps[:])

            # ---- Phase C: v projection -> vtmp in DRAM ----
            with (
                tc.tile_pool(name="wC", bufs=1) as wp,
                tc.tile_pool(name="xC", bufs=2) as xs,
                tc.tile_pool(name="evC", bufs=3) as ev,
                tc.tile_pool(name="psC", bufs=4, space="PSUM") as psp,
            ):
                wv = wp.tile([128, DC * D], f32r)
                nc.sync.dma_start(wv[:].rearrange("p (c o) -> p c o", c=DC),
                                  wv_d.rearrange("(c p) o -> p c o", p=128))
                for sg in range(4):
                    xkv = xs.tile([128, DC * 512], f32r, tag="xkv")
                    nc.sync.dma_start(
                        xkv[:].rearrange("p (c s) -> p c s", c=DC),
                        xkv_d[:, sg * 512:(sg + 1) * 512]
                        .rearrange("(c p) s -> p c s", p=128))
                    for st in range(4):
                        s0 = sg * 512 + st * 128
                        for og in range(2):
                            ps = psp.tile([128, 512], f32, tag="pj")
                            for dc in range(DC):
                                nc.tensor.matmul(
                                    ps[:],
                                    xkv[:, dc * 512 + st * 128:dc * 512 + st * 128 + 128],
                                    wv[:, dc * D + og * 512:dc * D + og * 512 + 512],
                                    start=(dc == 0), stop=(dc == DC - 1))
                            vt = ev.tile([128, 512], f32r, tag="ev")
                            nc.vector.tensor_copy(vt[:], ps[:])
                            nc.sync.dma_start(
                                vtmp[s0:s0 + 128, og * 512:og * 512 + 512],
                                vt[:])

            # ---- Phase D: attention ----
            with (
                tc.tile_pool(name="vD", bufs=1) as vp,
                tc.tile_pool(name="mD", bufs=2) as mp,
                tc.tile_pool(name="pD", bufs=1) as pp,
                tc.tile_pool(name="ptD", bufs=1) as ptp,
                tc.tile_pool(name="oD", bufs=2) as op,
                tc.tile_pool(name="smD", bufs=2) as smp,
                tc.tile_pool(name="psS", bufs=2, space="PSUM") as ps_s,
                tc.tile_pool(name="psT", bufs=2, space="PSUM") as ps_t,
                tc.tile_pool(name="psO", bufs=1, space="PSUM") as ps_o,
            ):
                vlo = vp.tile([128, 8 * D], f32r)  # kv blocks 0..7
                vhi = vp.tile([128, 8 * D], f32r)  # kv blocks 8..15
                nc.sync.dma_start(
                    vlo[:].rearrange("p (c o) -> p c o", c=8),
                    vtmp[0:1024, :].rearrange("(c p) o -> p c o", p=128))
                nc.sync.dma_start(
                    vhi[:].rearrange("p (c o) -> p c o", c=8),
                    vtmp[1024:2048, :].rearrange("(c p) o -> p c o", p=128))

                def vblk(kb):
                    t = vlo if kb < 8 else vhi
                    c = kb % 8
                    return t[:, c * D:(c + 1) * D]

                for j in range(NT):
                    E = _E(j)
                    kvlen = E * 512
                    mask = mp.tile([128, 512], f32r, tag="mask")
                    nc.sync.dma_start(mask[:],
                                      masks_d[j * 128:(j + 1) * 128, :])

                    P = pp.tile([128, NB * 128], f32r, tag="P")
                    dslots = smp.tile([128, 4], f32, tag="ds")
                    for g in range(E):
                        last = (g == E - 1)
                        sps = ps_s.tile([128, 512], f32, tag="sc")
                        for oc in range(DC):
                            nc.tensor.matmul(
                                sps[:],
                                qT[:, oc * 1024 + j * 128:oc * 1024 + j * 128 + 128],
                                kT[:, oc * S + g * 512:oc * S + g * 512 + 512],
                                start=(oc == 0),
                                stop=(oc == DC - 1 and not last))
                        if last:
                            nc.tensor.matmul(sps[:], ident[:], mask[:],
                                             start=False, stop=True)
                        nc.scalar.activation(
                            P[:, g * 512:(g + 1) * 512], sps[:], AF.Exp,
                            scale=float(SCALE),
                            accum_out=dslots[:, g:g + 1])
                    rcp = smp.tile([128, 1], f32, tag="rcp")
                    den = smp.tile([128, 1], f32, tag="den")
                    nc.vector.reduce_sum(den[:], dslots[:, 0:E],
                                         axis=mybir.AxisListType.X)
                    nc.vector.reciprocal(rcp[:], den[:])

                    PT = ptp.tile([128, NB * 128], f32r, tag="PT")
                    for g in range(E):
                        tps = ps_t.tile([128, 512], f32r, tag="tp")
                        for bb in range(4):
                            nc.tensor.transpose(
                                tps[:, bb * 128:(bb + 1) * 128],
                                P[:, g * 512 + bb * 128:g * 512 + bb * 128 + 128],
                                ident[:])
                        nc.vector.tensor_copy(PT[:, g * 512:(g + 1) * 512],
                                              tps[:])

                    ops = ps_o.tile([128, D], f32, tag="av")
                    nkb = 4 * E
                    for og in range(2):
                        for kb in range(nkb):
                            nc.tensor.matmul(
                                ops[:, og * 512:(og + 1) * 512],
                                PT[:, kb * 128:(kb + 1) * 128],
                                vblk(kb)[:, og * 512:(og + 1) * 512],
                                start=(kb == 0), stop=(kb == nkb - 1))
                    osb = op.tile([128, D], f32, tag="o")
                    nc.vector.tensor_scalar_mul(osb[:], ops[:], rcp[:])
                    nc.sync.dma_start(out_d[j * 128:(j + 1) * 128, :], osb[:])

    nc.compile()
    _cache["nc"] = nc
    return nc


def _shard(x, Wq, Wk, Wv):
    """Build the 8 per-core input maps from full inputs."""
    ident = np.eye(128, dtype=np.float32)
    wqt = np.ascontiguousarray(Wq.T)
    wkt = np.ascontiguousarray(Wk.T)
    wvt = np.ascontiguousarray(Wv.T)
    in_maps = []
    for c in range(8):
        b, p = c // 2, c % 2
        xb = x[b]
        xkv = np.ascontiguousarray(xb.T)
        rows = np.concatenate(
            [xb[(2 * j + p) * 128:(2 * j + p + 1) * 128, :] for j in range(8)],
            axis=0)
        xq = np.ascontiguousarray(rows.T)
        masks = np.empty((NT * 128, 512), np.float32)
        for j in range(NT):
            E = _E(j)
            q_abs = (2 * j + p) * 128 + np.arange(128)[:, None]
            kv_abs = (E - 1) * 512 + np.arange(512)[None, :]
            masks[j * 128:(j + 1) * 128, :] = np.where(kv_abs <= q_abs,
                                                       np.float32(0), NEG)
        in_maps.append({
            "xq": xq, "xkv": xkv, "wq": wqt, "wk": wkt, "wv": wvt,
            "masks": masks, "ident": ident,
        })
    return in_maps


def _unshard(results, dtype):
    out = np.empty((B, S, D), dtype)
    for c in range(8):
        b, p = c // 2, c % 2
        o = results[c]["out"]
        for j in range(NT):
            out[b, (2 * j + p) * 128:(2 * j + p + 1) * 128, :] = \
                o[j * 128:(j + 1) * 128, :]
    return out


def run(x, Wq, Wk, Wv, trace=False):
    from concourse.bass_utils import run_bass_kernel_spmd
    nc = _build()
    in_maps = _shard(np.asarray(x), np.asarray(Wq), np.asarray(Wk),
                     np.asarray(Wv))
    res = run_bass_kernel_spmd(nc, in_maps, core_ids=list(range(8)),
                               trace=trace)
    return _unshard(res.results, np.float32), res


def kernel(x, Wq, Wk, Wv):
    out, _ = run(x, Wq, Wk, Wv, trace=False)
    return out


# revision 5
# speedup vs baseline: 1.0120x; 1.0120x over previous
"""Causal self-attention (B=4, S=2048, D=1024, single head) on 8 TRN2 cores.

Sharding: core c handles batch b = c//2 with query-tile parity p = c%2 —
its 8 query tiles of 128 rows are the absolute 128-row tiles {2j+p}.
Interleaving parities balances causal work exactly: both cores of a pair
process kv extents of ceil((2j+2)/4) 512-wide groups per local tile j,
so the single SPMD program is uniform; per-core variation is data-only
(query rows and the additive causal mask for the last kv group).

All matmuls run in float32r (full-rate fp32 with reduced mantissa):
  qT[o,s] = WqT.T @ xT        kT likewise       v[s,o] = xT.T @ WvT
  scores[sq,kv] = qT.T @ kT (+ identity.T @ mask on the last group)
  P = exp(scale*scores) with fused row-sum (ScalarE accum_out)
  PT = PE-transpose(P);  out[sq,o] = PT.T @ v;  out *= 1/rowsum
"""

import numpy as np

B, S, D = 4, 2048, 1024
DC = D // 128          # contraction chunks
NB = S // 128          # kv blocks per batch
NT = 8                 # q tiles per core
SCALE = 1.0 / np.sqrt(np.float32(D))
NEG = np.float32(-1e30)

_cache = {}


def _E(j):
    # kv extent for local tile j, in 512-wide groups (uniform across cores)
    return (j + 2) // 2


def _build():
    if "nc" in _cache:
        return _cache["nc"]

    import concourse.bacc as bacc
    import concourse.mybir as mybir
    import concourse.tile as tile

    f32 = mybir.dt.float32
    f32r = mybir.dt.float32r
    AF = mybir.ActivationFunctionType

    nc = bacc.Bacc("TRN2", target_bir_lowering=False, debug=False,
                   num_devices=8)
    xq_d = nc.dram_tensor("xq", [D, NT * 128], f32r, kind="ExternalInput").ap()
    xkv_d = nc.dram_tensor("xkv", [D, S], f32r, kind="ExternalInput").ap()
    w_d = {n: nc.dram_tensor(n, [D, D], f32r, kind="ExternalInput").ap()
           for n in ("wq", "wk", "wv")}
    masks_d = nc.dram_tensor("masks", [NT * 128, 512], f32r,
                             kind="ExternalInput").ap()
    ident_d = nc.dram_tensor("ident", [128, 128], f32r,
                             kind="ExternalInput").ap()
    out_d = nc.dram_tensor("out", [NT * 128, D], f32,
                           kind="ExternalOutput").ap()

    with tile.TileContext(nc) as tc:
        with (
            tc.tile_pool(name="persist", bufs=1) as persist,
            tc.tile_pool(name="dram", bufs=1, space="DRAM") as dramp,
        ):
            kT = persist.tile([128, DC * S], f32r)          # [o%128, oc*S+kv]
            qT = persist.tile([128, DC * NT * 128], f32r)   # [o%128, oc*1024+sq]
            vtmp = [dramp.tile([512, D], f32r, name=f"vtmp{i}",
                                tag=f"vtmp{i}") for i in range(4)]

            with (
                tc.tile_pool(name="wpool", bufs=2) as wp,
                tc.tile_pool(name="xpool", bufs=2) as xs,
                tc.tile_pool(name="pspj", bufs=4, space="PSUM") as psp,
                tc.tile_pool(name="evpj", bufs=3) as ev,
            ):
                def load_w(name):
                    w = wp.tile([128, DC * D], f32r, tag="w")  # [d%128, dc*D+o]
                    src = w_d[name].rearrange("(c p) o -> p c o", p=128)
                    for dc in range(DC):
                        nc.sync.dma_start(
                            w[:].rearrange("p (c o) -> p c o", c=DC)[:, dc],
                            src[:, dc])
                    return w

                def load_x(src_ap, c0):
                    xt = xs.tile([128, DC * 256], f32r, tag="x")
                    nc.sync.dma_start(
                        xt[:].rearrange("p (c s) -> p c s", c=DC),
                        src_ap[:, c0 * 256:(c0 + 1) * 256]
                        .rearrange("(c p) s -> p c s", p=128))
                    return xt

                wq = load_w("wq")
                wk = load_w("wk")

                # ---- Phase A: q projection (into resident qT) ----
                for sg in range(4):
                    xt = load_x(xq_d, sg)
                    for ot in range(8):
                        ps = psp.tile([128, 256], f32, tag="pj")
                        for dc in range(DC):
                            nc.tensor.matmul(
                                ps[:],
                                wq[:, dc * D + ot * 128:dc * D + ot * 128 + 128],
                                xt[:, dc * 256:(dc + 1) * 256],
                                start=(dc == 0), stop=(dc == DC - 1))
                        nc.vector.tensor_copy(
                            qT[:, ot * 1024 + sg * 256:ot * 1024 + sg * 256 + 256],
                            ps[:])

                wv = load_w("wv")  # reuses wq's slot; prefetches during B

                # ---- Phase B: k projection (into resident kT) ----
                for sg in range(8):
                    xt = load_x(xkv_d, sg)
                    for ot in range(8):
                        ps = psp.tile([128, 256], f32, tag="pj")
                        for dc in range(DC):
                            nc.tensor.matmul(
                                ps[:],
                                wk[:, dc * D + ot * 128:dc * D + ot * 128 + 128],
                                xt[:, dc * 256:(dc + 1) * 256],
                                start=(dc == 0), stop=(dc == DC - 1))
                        nc.vector.tensor_copy(
                            kT[:, ot * S + sg * 256:ot * S + sg * 256 + 256],
                            ps[:])

                # ---- Phase C: v projection -> vtmp DRAM (4 chunks) ----
                for sg in range(8):
                    xt = load_x(xkv_d, sg)
                    for st in range(2):
                        s0 = sg * 256 + st * 128
                        vt_dram = vtmp[s0 // 512]
                        for og in range(2):
                            ps = psp.tile([128, 512], f32, tag="pj")
                            for dc in range(DC):
                                nc.tensor.matmul(
                                    ps[:],
                                    xt[:, dc * 256 + st * 128:dc * 256 + st * 128 + 128],
                                    wv[:, dc * D + og * 512:dc * D + og * 512 + 512],
                                    start=(dc == 0), stop=(dc == DC - 1))
                            vtb = ev.tile([128, 512], f32r, tag="ev")
                            nc.vector.tensor_copy(vtb[:], ps[:])
                            nc.sync.dma_start(
                                vt_dram[s0 % 512:s0 % 512 + 128,
                                        og * 512:og * 512 + 512],
                                vtb[:])

            # ---- Phase D: attention ----
            with (
                tc.tile_pool(name="vD", bufs=1) as vp,
                tc.tile_pool(name="cD", bufs=1) as cp,
                tc.tile_pool(name="mD", bufs=2) as mp,
                tc.tile_pool(name="pD", bufs=1) as pp,
                tc.tile_pool(name="ptD", bufs=1) as ptp,
                tc.tile_pool(name="oD", bufs=2) as op,
                tc.tile_pool(name="smD", bufs=2) as smp,
                tc.tile_pool(name="psS", bufs=2, space="PSUM") as ps_s,
                tc.tile_pool(name="psT", bufs=2, space="PSUM") as ps_t,
                tc.tile_pool(name="psO", bufs=1, space="PSUM") as ps_o,
            ):
                ident = cp.tile([128, 128], f32r)
                nc.sync.dma_start(ident[:], ident_d)
                vsb = []
                for q4 in range(4):  # kv blocks 4q4..4q4+3
                    t = vp.tile([128, 4 * D], f32r, name=f"vsb{q4}", tag=f"v{q4}")
                    nc.sync.dma_start(
                        t[:].rearrange("p (c o) -> p c o", c=4),
                        vtmp[q4][:].rearrange("(c p) o -> p c o", p=128))
                    vsb.append(t)

                def vblk(kb):
                    return vsb[kb // 4][:, (kb % 4) * D:(kb % 4 + 1) * D]

                for j in range(NT):
                    E = _E(j)
                    mask = mp.tile([128, 512], f32r, tag="mask")
                    nc.sync.dma_start(mask[:],
                                      masks_d[j * 128:(j + 1) * 128, :])

                    P = pp.tile([128, NB * 128], f32r, tag="P")
                    dslots = smp.tile([128, 4], f32, tag="ds")
                    for g in range(E):
                        last = (g == E - 1)
                        sps = ps_s.tile([128, 512], f32, tag="sc")
                        for oc in range(DC):
                            nc.tensor.matmul(
                                sps[:],
                                qT[:, oc * 1024 + j * 128:oc * 1024 + j * 128 + 128],
                                kT[:, oc * S + g * 512:oc * S + g * 512 + 512],
                                start=(oc == 0),
                                stop=(oc == DC - 1 and not last))
                        if last:
                            nc.tensor.matmul(sps[:], ident[:], mask[:],
                                             start=False, stop=True)
                        nc.scalar.activation(
                            P[:, g * 512:(g + 1) * 512], sps[:], AF.Exp,
                            scale=float(SCALE),
                            accum_out=dslots[:, g:g + 1])
                    rcp = smp.tile([128, 1], f32, tag="rcp")
                    den = smp.tile([128, 1], f32, tag="den")
                    nc.vector.reduce_sum(den[:], dslots[:, 0:E],
                                         axis=mybir.AxisListType.X)
                    nc.vector.reciprocal(rcp[:], den[:])

                    PT = ptp.tile([128, NB * 128], f32r, tag="PT")
                    for g in range(E):
                        tps = ps_t.tile([128, 512], f32r, tag="tp")
                        for bb in range(4):
                            nc.tensor.transpose(
                                tps[:, bb * 128:(bb + 1) * 128],
                                P[:, g * 512 + bb * 128:g * 512 + bb * 128 + 128],
                                ident[:])
                        nc.vector.tensor_copy(PT[:, g * 512:(g + 1) * 512],
                                              tps[:])

                    ops = ps_o.tile([128, D], f32, tag="av")
                    nkb = 4 * E
                    for og in range(2):
                        for kb in range(nkb):
                            nc.tensor.matmul(
                                ops[:, og * 512:(og + 1) * 512],
                                PT[:, kb * 128:(kb + 1) * 128],
                                vblk(kb)[:, og * 512:(og + 1) * 512],
                                start=(kb == 0), stop=(kb == nkb - 1))
                    osb = op.tile([128, D], f32, tag="o")
                    nc.vector.tensor_scalar_mul(osb[:], ops[:], rcp[:])
                    nc.sync.dma_start(out_d[j * 128:(j + 1) * 128, :], osb[:])

    nc.compile()
    _cache["nc"] = nc
    return nc


def _shard(x, Wq, Wk, Wv):
    """Build the 8 per-core input maps from full inputs."""
    ident = np.eye(128, dtype=np.float32)
    wqt = np.ascontiguousarray(Wq.T)
    wkt = np.ascontiguousarray(Wk.T)
    wvt = np.ascontiguousarray(Wv.T)
    in_maps = []
    for c in range(8):
        b, p = c // 2, c % 2
        xb = x[b]
        xkv = np.ascontiguousarray(xb.T)
        rows = np.concatenate(
            [xb[(2 * j + p) * 128:(2 * j + p + 1) * 128, :] for j in range(8)],
            axis=0)
        xq = np.ascontiguousarray(rows.T)
        masks = np.empty((NT * 128, 512), np.float32)
        for j in range(NT):
            E = _E(j)
            q_abs = (2 * j + p) * 128 + np.arange(128)[:, None]
            kv_abs = (E - 1) * 512 + np.arange(512)[None, :]
            masks[j * 128:(j + 1) * 128, :] = np.where(kv_abs <= q_abs,
                                                       np.float32(0), NEG)
        in_maps.append({
            "xq": xq, "xkv": xkv, "wq": wqt, "wk": wkt, "wv": wvt,
            "masks": masks, "ident": ident,
        })
    return in_maps


def _unshard(results, dtype):
    out = np.empty((B, S, D), dtype)
    for c in range(8):
        b, p = c // 2, c % 2
        o = results[c]["out"]
        for j in range(NT):
            out[b, (2 * j + p) * 128:(2 * j + p + 1) * 128, :] = \
                o[j * 128:(j + 1) * 128, :]
    return out


def run(x, Wq, Wk, Wv, trace=False):
    from concourse.bass_utils import run_bass_kernel_spmd
    nc = _build()
    in_maps = _shard(np.asarray(x), np.asarray(Wq), np.asarray(Wk),
                     np.asarray(Wv))
    res = run_bass_kernel_spmd(nc, in_maps, core_ids=list(range(8)),
                               trace=trace)
    return _unshard(res.results, np.float32), res


def kernel(x, Wq, Wk, Wv):
    out, _ = run(x, Wq, Wk, Wv, trace=False)
    return out


# revision 6
# speedup vs baseline: 1.2302x; 1.2156x over previous
"""Causal self-attention (B=4, S=2048, D=1024, single head) on 8 TRN2 cores.

Sharding: core c handles batch b = c//2 with query-tile parity p = c%2 —
its 8 query tiles of 128 rows are the absolute 128-row tiles {2j+p}.
Interleaving parities balances causal work exactly: both cores of a pair
process kv extents of ceil((2j+2)/4) 512-wide groups per local tile j,
so the single SPMD program is uniform; per-core variation is data-only
(query rows and the additive causal mask for the last kv group).

All matmuls run in float32r (full-rate fp32 with reduced mantissa):
  qT[o,s] = WqT.T @ xT        kT likewise       v[s,o] = xT.T @ WvT
  scores[sq,kv] = qT.T @ kT (+ identity.T @ mask on the last group)
  P = exp(scale*scores) with fused row-sum (ScalarE accum_out)
  PT = PE-transpose(P);  out[sq,o] = PT.T @ v;  out *= 1/rowsum
"""

import numpy as np

B, S, D = 4, 2048, 1024
DC = D // 128          # contraction chunks
NB = S // 128          # kv blocks per batch
NT = 8                 # q tiles per core
SCALE = 1.0 / np.sqrt(np.float32(D))
NEG = np.float32(-1e30)

_cache = {}


def _E(j):
    # kv extent for local tile j, in 512-wide groups (uniform across cores)
    return (j + 2) // 2


def _build():
    if "nc" in _cache:
        return _cache["nc"]

    import concourse.bacc as bacc
    import concourse.mybir as mybir
    import concourse.tile as tile

    f32 = mybir.dt.float32
    f32r = mybir.dt.float32r
    AF = mybir.ActivationFunctionType

    nc = bacc.Bacc("TRN2", target_bir_lowering=False, debug=False,
                   num_devices=8)
    xq_d = nc.dram_tensor("xq", [D, NT * 128], f32r, kind="ExternalInput").ap()
    xkv_d = nc.dram_tensor("xkv", [D, S], f32r, kind="ExternalInput").ap()
    w_d = {n: nc.dram_tensor(n, [D, D], f32r, kind="ExternalInput").ap()
           for n in ("wq", "wk", "wv")}
    masks_d = nc.dram_tensor("masks", [NT * 128, 512], f32r,
                             kind="ExternalInput").ap()
    ident_d = nc.dram_tensor("ident", [128, 128], f32r,
                             kind="ExternalInput").ap()
    out_d = nc.dram_tensor("out", [NT * 128, D], f32,
                           kind="ExternalOutput").ap()

    with tile.TileContext(nc) as tc:
        with (
            tc.tile_pool(name="persist", bufs=1) as persist,
            tc.tile_pool(name="dram", bufs=1, space="DRAM") as dramp,
        ):
            kT = persist.tile([128, DC * S], f32r)          # [o%128, oc*S+kv]
            qT = persist.tile([128, DC * NT * 128], f32r)   # [o%128, oc*1024+sq]
            vtmp = [dramp.tile([512, D], f32r, name=f"vtmp{i}",
                               tag=f"vtmp{i}") for i in range(4)]

            with (
                tc.tile_pool(name="wpool", bufs=2) as wp,
                tc.tile_pool(name="xpool", bufs=2) as xs,
                tc.tile_pool(name="evpool", bufs=3) as ev,
                tc.tile_pool(name="pspj", bufs=4, space="PSUM") as psp,
            ):
                def load_w(name):
                    # two half DMAs so the first d-chunks arrive early
                    w = wp.tile([128, DC * D], f32r, name=f"w_{name}",
                                tag="w")  # [d%128, dc*D + o]
                    src = w_d[name].rearrange("(c p) o -> p c o", p=128)
                    wv3 = w[:].rearrange("p (c o) -> p c o", c=DC)
                    nc.sync.dma_start(wv3[:, 0:DC // 2], src[:, 0:DC // 2])
                    nc.sync.dma_start(wv3[:, DC // 2:DC], src[:, DC // 2:DC])
                    return w

                def load_x(src_ap, c0):
                    xt = xs.tile([128, DC * 512], f32r, tag="x")
                    nc.sync.dma_start(
                        xt[:].rearrange("p (c s) -> p c s", c=DC),
                        src_ap[:, c0 * 512:(c0 + 1) * 512]
                        .rearrange("(c p) s -> p c s", p=128))
                    return xt

                wq = load_w("wq")
                xt0 = load_x(xq_d, 0)
                wk = load_w("wk")

                # ---- Phase A: q projection (into resident qT) ----
                for sg in range(2):
                    xt = xt0 if sg == 0 else load_x(xq_d, sg)
                    for ot in range(8):
                        ps = psp.tile([128, 512], f32, tag="pj")
                        for dc in range(DC):
                            nc.tensor.matmul(
                                ps[:],
                                wq[:, dc * D + ot * 128:dc * D + ot * 128 + 128],
                                xt[:, dc * 512:(dc + 1) * 512],
                                start=(dc == 0), stop=(dc == DC - 1))
                        nc.vector.tensor_copy(
                            qT[:, ot * 1024 + sg * 512:ot * 1024 + sg * 512 + 512],
                            ps[:])

                wv = load_w("wv")  # takes wq's slot; prefetches during BC

                # ---- Phase BC: k and v projections from shared x chunks ----
                for sg in range(4):
                    xt = load_x(xkv_d, sg)
                    for ot in range(8):
                        ps = psp.tile([128, 512], f32, tag="pj")
                        for dc in range(DC):
                            nc.tensor.matmul(
                                ps[:],
                                wk[:, dc * D + ot * 128:dc * D + ot * 128 + 128],
                                xt[:, dc * 512:(dc + 1) * 512],
                                start=(dc == 0), stop=(dc == DC - 1))
                        nc.vector.tensor_copy(
                            kT[:, ot * S + sg * 512:ot * S + sg * 512 + 512],
                            ps[:])
                    for st in range(4):
                        for og in range(2):
                            ps = psp.tile([128, 512], f32, tag="pj")
                            for dc in range(DC):
                                nc.tensor.matmul(
                                    ps[:],
                                    xt[:, dc * 512 + st * 128:dc * 512 + st * 128 + 128],
                                    wv[:, dc * D + og * 512:dc * D + og * 512 + 512],
                                    start=(dc == 0), stop=(dc == DC - 1))
                            vtb = ev.tile([128, 512], f32r, tag="ev")
                            nc.vector.tensor_copy(vtb[:], ps[:])
                            nc.gpsimd.dma_start(
                                vtmp[sg][st * 128:st * 128 + 128,
                                         og * 512:og * 512 + 512],
                                vtb[:])

            # ---- Phase D: attention ----
            with (
                tc.tile_pool(name="vD", bufs=1) as vp,
                tc.tile_pool(name="cD", bufs=1) as cp,
                tc.tile_pool(name="pD", bufs=1) as pp,
                tc.tile_pool(name="ptD", bufs=1) as ptp,
                tc.tile_pool(name="oD", bufs=2) as op,
                tc.tile_pool(name="smD", bufs=2) as smp,
                tc.tile_pool(name="psS", bufs=3, space="PSUM") as ps_s,
                tc.tile_pool(name="psT", bufs=2, space="PSUM") as ps_t,
                tc.tile_pool(name="psO", bufs=1, space="PSUM") as ps_o,
            ):
                masks = cp.tile([128, NT * 512], f32r)  # [p, j*512+kv]
                nc.sync.dma_start(
                    masks[:].rearrange("p (j k) -> p j k", j=NT),
                    masks_d.rearrange("(j p) k -> p j k", p=128))
                ident = cp.tile([128, 128], f32r)
                nc.sync.dma_start(ident[:], ident_d)
                vsb = []
                for q4 in range(4):  # kv blocks 4q4..4q4+3
                    t = vp.tile([128, 4 * D], f32r, name=f"vsb{q4}",
                                tag=f"v{q4}")
                    nc.sync.dma_start(
                        t[:].rearrange("p (c o) -> p c o", c=4),
                        vtmp[q4][:].rearrange("(c p) o -> p c o", p=128))
                    vsb.append(t)

                def vblk(kb):
                    return vsb[kb // 4][:, (kb % 4) * D:(kb % 4 + 1) * D]

                for j in range(NT):
                    E = _E(j)
                    P = pp.tile([128, NB * 128], f32r, tag="P")
                    dslots = smp.tile([128, 4], f32, tag="ds")
                    for g in range(E):
                        last = (g == E - 1)
                        sps = ps_s.tile([128, 512], f32, tag="sc")
                        for oc in range(DC):
                            nc.tensor.matmul(
                                sps[:],
                                qT[:, oc * 1024 + j * 128:oc * 1024 + j * 128 + 128],
                                kT[:, oc * S + g * 512:oc * S + g * 512 + 512],
                                start=(oc == 0),
                                stop=(oc == DC - 1 and not last))
                        if last:
                            nc.tensor.matmul(
                                sps[:], ident[:],
                                masks[:, j * 512:(j + 1) * 512],
                                start=False, stop=True)
                        nc.scalar.activation(
                            P[:, g * 512:(g + 1) * 512], sps[:], AF.Exp,
                            scale=float(SCALE),
                            accum_out=dslots[:, g:g + 1])
                    rcp = smp.tile([128, 1], f32, tag="rcp")
                    den = smp.tile([128, 1], f32, tag="den")
                    nc.vector.reduce_sum(den[:], dslots[:, 0:E],
                                         axis=mybir.AxisListType.X)
                    nc.vector.reciprocal(rcp[:], den[:])

                    PT = ptp.tile([128, NB * 128], f32r, tag="PT")
                    for g in range(E):
                        tps = ps_t.tile([128, 512], f32r, tag="tp")
                        for bb in range(4):
                            nc.tensor.transpose(
                                tps[:, bb * 128:(bb + 1) * 128],
                                P[:, g * 512 + bb * 128:g * 512 + bb * 128 + 128],
                                ident[:])
                        nc.vector.tensor_copy(PT[:, g * 512:(g + 1) * 512],
                                              tps[:])

                    ops = ps_o.tile([128, D], f32, tag="av")
                    nkb = 4 * E
                    for og in range(2):
                        for kb in range(nkb):
                            nc.tensor.matmul(
                                ops[:, og * 512:(og + 1) * 512],
                                PT[:, kb * 128:(kb + 1) * 128],
                                vblk(kb)[:, og * 512:(og + 1) * 512],
                                start=(kb == 0), stop=(kb == nkb - 1))
                    osb = op.tile([128, D], f32, tag="o")
                    nc.vector.tensor_scalar_mul(osb[:], ops[:], rcp[:])
                    nc.sync.dma_start(out_d[j * 128:(j + 1) * 128, :], osb[:])

    nc.compile()
    _cache["nc"] = nc
    return nc


def _shard(x, Wq, Wk, Wv):
    """Build the 8 per-core input maps from full inputs."""
    ident = np.eye(128, dtype=np.float32)
    wqt = np.ascontiguousarray(Wq.T)
    wkt = np.ascontiguousarray(Wk.T)
    wvt = np.ascontiguousarray(Wv.T)
    in_maps = []
    for c in range(8):
        b, p = c // 2, c % 2
        xb = x[b]
        xkv = np.ascontiguousarray(xb.T)
        rows = np.concatenate(
            [xb[(2 * j + p) * 128:(2 * j + p + 1) * 128, :] for j in range(8)],
            axis=0)
        xq = np.ascontiguousarray(rows.T)
        masks = np.empty((NT * 128, 512), np.float32)
        for j in range(NT):
            E = _E(j)
            q_abs = (2 * j + p) * 128 + np.arange(128)[:, None]
            kv_abs = (E - 1) * 512 + np.arange(512)[None, :]
            masks[j * 128:(j + 1) * 128, :] = np.where(kv_abs <= q_abs,
                                                       np.float32(0), NEG)
        in_maps.append({
            "xq": xq, "xkv": xkv, "wq": wqt, "wk": wkt, "wv": wvt,
            "masks": masks, "ident": ident,
        })
    return in_maps


def _unshard(results, dtype):
    out = np.empty((B, S, D), dtype)
    for c in range(8):
        b, p = c // 2, c % 2
        o = results[c]["out"]
        for j in range(NT):
            out[b, (2 * j + p) * 128:(2 * j + p + 1) * 128, :] = \
                o[j * 128:(j + 1) * 128, :]
    return out


def run(x, Wq, Wk, Wv, trace=False):
    from concourse.bass_utils import run_bass_kernel_spmd
    nc = _build()
    in_maps = _shard(np.asarray(x), np.asarray(Wq), np.asarray(Wk),
                     np.asarray(Wv))
    res = run_bass_kernel_spmd(nc, in_maps, core_ids=list(range(8)),
                               trace=trace)
    return _unshard(res.results, np.float32), res


def kernel(x, Wq, Wk, Wv):
    out, _ = run(x, Wq, Wk, Wv, trace=False)
    return out
